# revision 17
# baseline (speedup 1.0000x reference)
"""ContrastMaximization kernel for Trainium2 (8 NeuronCores).

Strategy (data-parallel over batch x event-chunks):
  - core c handles batch c//4, event quarter c%4 (65536 events).
  - host sorts each shard by temporal bin zi=floor(t) and pads each z-group
    to a fixed number of 128-event tiles (dummy events have zero weight).
  - Pass A (per event tile): bilinear flow gather via matmul:
      W1[ev,(c,x)] = sum_y neghat_y[y,ev] * flow_z[y,(c,x)]   (PE, f32r)
      u[ev,c]     = sum_x neghat_x[ev,x] * W1[ev,(c,x)]       (DVE fused ttr)
    then per-event warp params and warped coords for all 11 references.
  - Pass B (hw loop over r): bilinear scatter-add via one-hot matmuls:
      grids[pol,{iwe,iwt},yh][y,x] += neghat_y^T @ neghat_x    (PSUM accum)
  - per-r grids -> DRAM, AllReduce over the 4 cores of the batch, then the
    contrast loss (iwat^2 sums + inside counts) on-device -> loss[1,11].
"""

import numpy as np
from contextlib import ExitStack

import concourse.bass as bass
import concourse.bacc as bacc
import concourse.tile as tile
import concourse.mybir as mybir
from concourse.bass_utils import run_bass_kernel_spmd

AluOp = mybir.AluOpType
AF = mybir.ActivationFunctionType
FP32 = mybir.dt.float32
F32R = mybir.dt.float32r
AX = mybir.AxisListType

B, N, BASE, H, W = 2, 262144, 10, 256, 256
R = BASE + 1
NCORES = 8
CORES_PER_B = 4
SHARD = N // CORES_PER_B          # events per core
EPS = 1e-9
XW = 514                          # x-hat width: 2 pol sections of 257
YW = 258                          # y-hat width
RW = 12                           # padded r width (even)


def build_kernel(nc, zg, do_allreduce, ngroup, out_grids=False, use_for_i=True,
                 phases="ABL"):
    """zg: tiles per z-group. ngroup: cores per reduce group."""
    nt = BASE * zg                # event tiles per core

    def din(name, shape):
        return nc.dram_tensor(name, shape, FP32, kind="ExternalInput")

    ev_x = din("ev_x", [128, nt])
    ev_y = din("ev_y", [128, nt])
    ev_ts = din("ev_ts", [128, nt])
    ev_negt = din("ev_negt", [128, nt])
    ev_xoff = din("ev_xoff", [128, nt])      # x + 257*p
    ev_pm1 = din("ev_pm1", [128, nt])        # 257*p - 1
    ev_pp256 = din("ev_pp256", [128, nt])    # 257*p + 256
    flow_in = din("flow_in", [128, 2 * BASE, 512])   # [ylane,(z,yh),(c,x)]
    c_iota514 = din("c_iota514", [128, XW])
    c_iota258 = din("c_iota258", [128, YW])
    c_r12 = din("c_r12", [128, RW])
    c_ident = din("c_ident", [128, 128])
    c_ones = din("c_ones", [128, 2])

    out_loss = nc.dram_tensor("loss", [1, R], FP32, kind="ExternalOutput")
    # grids dram: [r, grid(iwe/iwt), pol, yh, ylane, x]
    gshape = [R, 2, 2, 2, 128, 256]
    if out_grids:
        gr_loc = nc.dram_tensor("grids", gshape, FP32, kind="ExternalOutput")
    else:
        gr_loc = nc.dram_tensor("gr_loc", gshape, FP32)
    if do_allreduce:
        gr_red = nc.dram_tensor("gr_red", gshape, FP32)
    else:
        gr_red = gr_loc

    with tile.TileContext(nc) as tc:
        with ExitStack() as ctx:
            cpool = ctx.enter_context(tc.tile_pool(name="consts", bufs=1))
            ppool = ctx.enter_context(tc.tile_pool(name="params", bufs=1))

            # resident inputs / consts
            t_iota514 = cpool.tile([128, XW], FP32)
            t_iota258 = cpool.tile([128, YW], FP32)
            t_r12 = cpool.tile([128, RW], FP32)
            t_ident = cpool.tile([128, 128], FP32)
            t_ones = cpool.tile([128, 2], FP32)
            t_flow = cpool.tile([128, 2 * BASE, 512], F32R)
            nc.sync.dma_start(t_iota514[:], c_iota514[:])
            nc.sync.dma_start(t_iota258[:], c_iota258[:])
            nc.sync.dma_start(t_r12[:], c_r12[:])
            nc.sync.dma_start(t_ident[:], c_ident[:])
            nc.sync.dma_start(t_ones[:], c_ones[:])
            nc.gpsimd.dma_start(t_flow[:], flow_in[:])   # casts f32 -> f32r

            t_x = ppool.tile([128, nt], FP32)
            t_y = ppool.tile([128, nt], FP32)
            t_ts = ppool.tile([128, nt], FP32)
            t_negt = ppool.tile([128, nt], FP32)
            t_xoff = ppool.tile([128, nt], FP32)
            t_pm1 = ppool.tile([128, nt], FP32)
            t_pp256 = ppool.tile([128, nt], FP32)
            nc.sync.dma_start(t_x[:], ev_x[:])
            nc.sync.dma_start(t_y[:], ev_y[:])
            nc.sync.dma_start(t_ts[:], ev_ts[:])
            nc.sync.dma_start(t_negt[:], ev_negt[:])
            nc.sync.dma_start(t_xoff[:], ev_xoff[:])
            nc.sync.dma_start(t_pm1[:], ev_pm1[:])
            nc.sync.dma_start(t_pp256[:], ev_pp256[:])

            # outputs of pass A
            t_ux = ppool.tile([128, nt], FP32)
            t_uy = ppool.tile([128, nt], FP32)
            t_ax = ppool.tile([128, nt], FP32)
            t_ay = ppool.tile([128, nt], FP32)

            # ---------------- pass A: flow gather + warp params ----------------
            with ExitStack() as actx:
                apool = actx.enter_context(tc.tile_pool(name="pA", bufs=3))
                apsum = actx.enter_context(
                    tc.tile_pool(name="pAps", bufs=2, space="PSUM"))

                for t in range(nt):
                    z = t // zg
                    xc = t_x[:, t:t + 1]
                    yc = t_y[:, t:t + 1]

                    dyA = apool.tile([128, YW], FP32, tag="dyA")
                    nc.scalar.activation(dyA[:], t_iota258[:], AF.Abs,
                                         bias=yc, scale=-1.0)
                    hyA = apool.tile([128, YW], FP32, tag="hyA")
                    nc.vector.tensor_scalar(hyA[:], dyA[:], 1.0, 0.0,
                                            AluOp.subtract, AluOp.min)
                    # transpose both 128-chunks -> [y, ev]
                    psT = apsum.tile([128, 256], FP32, tag="psT")
                    nc.tensor.transpose(psT[:, 0:128], hyA[:, 0:128], t_ident[:])
                    nc.tensor.transpose(psT[:, 128:256], hyA[:, 128:256], t_ident[:])
                    hyT = apool.tile([128, 256], F32R, tag="hyT")
                    nc.vector.tensor_copy(hyT[:, 0:128], psT[:, 0:128])
                    nc.vector.tensor_copy(hyT[:, 128:256], psT[:, 128:256])

                    W1 = apsum.tile([128, 512], FP32, tag="W1")
                    nc.tensor.matmul(W1[:], hyT[:, 0:128],
                                     t_flow[:, 2 * z, :], start=True, stop=False)
                    nc.tensor.matmul(W1[:], hyT[:, 128:256],
                                     t_flow[:, 2 * z + 1, :], start=False, stop=True)

                    dxA = apool.tile([128, YW], FP32, tag="dxA")
                    nc.scalar.activation(dxA[:], t_iota258[:], AF.Abs,
                                         bias=xc, scale=-1.0)
                    hxA = apool.tile([128, YW], FP32, tag="hxA")
                    nc.vector.tensor_scalar(hxA[:], dxA[:], 1.0, 0.0,
                                            AluOp.subtract, AluOp.min)

                    scr = apool.tile([128, 512], FP32, tag="scr")
                    hxA2 = bass.AP(tensor=hxA.tensor, offset=hxA.offset,
                                   ap=[hxA.ap[0], [0, 2], [1, 256]])
                    nc.vector.tensor_tensor(
                        scr[:].rearrange("e (c x) -> e c x", c=2),
                        W1[:].rearrange("e (c x) -> e c x", c=2),
                        hxA2, AluOp.mult)
                    nc.vector.tensor_reduce(t_ux[:, t:t + 1], scr[:, 0:256],
                                            axis=AX.X, op=AluOp.add)
                    nc.vector.tensor_reduce(t_uy[:, t:t + 1], scr[:, 256:512],
                                            axis=AX.X, op=AluOp.add)

                    # ax = xoff - t*ux ; ay = y - t*uy
                    nc.vector.tensor_scalar(t_ax[:, t:t + 1], t_ux[:, t:t + 1],
                                            t_negt[:, t:t + 1], t_xoff[:, t:t + 1],
                                            AluOp.mult, AluOp.add)
                    nc.vector.tensor_scalar(t_ay[:, t:t + 1], t_uy[:, t:t + 1],
                                            t_negt[:, t:t + 1], yc,
                                            AluOp.mult, AluOp.add)

            # ---------------- pass B: scatter-add histograms ----------------
            if "B" not in phases:
                z_ = ppool.tile([1, R], FP32, name="z_")
                nc.vector.memset(z_[:], 0.0)
                nc.sync.dma_start(out_loss[:], z_[:])
                return nc
            with ExitStack() as bctx:
                bpool = bctx.enter_context(tc.tile_pool(name="pB", bufs=3))
                bpsum = bctx.enter_context(
                    tc.tile_pool(name="pBps", bufs=1, space="PSUM"))
                spool = bctx.enter_context(tc.tile_pool(name="pBst", bufs=2))

                # 8 accumulators: [yh][grid][pol] -> [128, 256]
                acc = [bpsum.tile([128, 256], FP32, tag=f"acc{k}",
                                  name=f"acc{k}")
                       for k in range(8)]

                # running warped coords, advanced by +u each r
                wx_run = ppool.tile([128, nt], FP32, name="wx_run")
                wy_run = ppool.tile([128, nt], FP32, name="wy_run")
                nc.vector.tensor_copy(wx_run[:], t_ax[:])
                nc.vector.tensor_copy(wy_run[:], t_ay[:])

                def emit_r(r_idx):
                    wxc = bpool.tile([128, nt], FP32, tag="wxc")
                    nc.vector.tensor_tensor(wxc[:], wx_run[:], t_pm1[:],
                                            AluOp.max)
                    nc.vector.tensor_tensor(wxc[:], wxc[:], t_pp256[:],
                                            AluOp.min)
                    for t in range(nt):
                        dx = bpool.tile([128, XW], FP32, tag="dx")
                        nc.scalar.activation(dx[:], t_iota514[:], AF.Abs,
                                             bias=wxc[:, t:t + 1], scale=-1.0)
                        hx = bpool.tile([128, XW], F32R, tag="hx")
                        nc.vector.tensor_scalar(hx[:], dx[:], 1.0, 0.0,
                                                AluOp.subtract, AluOp.min)
                        hxt = bpool.tile([128, XW], F32R, tag="hxt")
                        nc.vector.tensor_scalar(hxt[:], hx[:], t_ts[:, t:t + 1],
                                                None, AluOp.mult)
                        dy = bpool.tile([128, YW], FP32, tag="dy")
                        nc.scalar.activation(dy[:], t_iota258[:], AF.Abs,
                                             bias=wy_run[:, t:t + 1], scale=-1.0)
                        hy = bpool.tile([128, YW], F32R, tag="hy")
                        nc.vector.tensor_scalar(hy[:], dy[:], 1.0, 0.0,
                                                AluOp.subtract, AluOp.min)
                        st = (t == 0)
                        sp = (t == nt - 1)
                        for h in range(2):
                            lhs = hy[:, 128 * h:128 * (h + 1)]
                            for g, rhs_t in ((0, hx), (1, hxt)):
                                for p in range(2):
                                    sec = rhs_t[:, 257 * p:257 * p + 256]
                                    nc.tensor.matmul(acc[h * 4 + g * 2 + p][:],
                                                     lhs, sec, start=st, stop=sp)
                    # copy out + advance
                    stage = spool.tile([128, 2048], FP32, tag="stage")
                    for h in range(2):
                        for g in range(2):
                            for p in range(2):
                                k = h * 4 + g * 2 + p
                                cs = slice(k * 256, (k + 1) * 256)
                                nc.scalar.copy(stage[:, cs], acc[k][:])
                                nc.gpsimd.dma_start(
                                    gr_loc[r_idx, g, p, h], stage[:, cs])
                    nc.vector.tensor_tensor(wx_run[:], wx_run[:], t_ux[:],
                                            AluOp.add)
                    nc.vector.tensor_tensor(wy_run[:], wy_run[:], t_uy[:],
                                            AluOp.add)

                if use_for_i:
                    with tc.For_i(0, R) as r_iv:
                        rbase = nc.snap(r_iv)
                        emit_r(bass.ds(rbase, 1))
                else:
                    for r in range(R):
                        emit_r(r)

            # ---------------- all-reduce ----------------
            if do_allreduce:
                groups = [list(range(g * ngroup, (g + 1) * ngroup))
                          for g in range(NCORES // ngroup)]
                nc.gpsimd.collective_compute(
                    "AllReduce", AluOp.add, replica_groups=groups,
                    ins=[gr_loc[:]], outs=[gr_red[:]])

            # ---------------- contrast loss ----------------
            if "L" not in phases:
                z_ = ppool.tile([1, R], FP32, name="z_")
                nc.vector.memset(z_[:], 0.0)
                nc.sync.dma_start(out_loss[:], z_[:])
                return nc
            with ExitStack() as lctx:
                lpool = lctx.enter_context(tc.tile_pool(name="loss", bufs=2))
                lacc = lctx.enter_context(tc.tile_pool(name="lacc", bufs=1))
                lpsum = lctx.enter_context(
                    tc.tile_pool(name="lps", bufs=1, space="PSUM"))

                t_acc = lacc.tile([128, 2 * RW], FP32)   # [0:R]=sq, [RW:RW+R]=ins
                for r in range(R):
                    sb = lpool.tile([128, 2048], FP32, tag="sb")
                    for g in range(2):
                        for p in range(2):
                            src = gr_red[r, g, p].rearrange("h y x -> y h x")
                            cs = slice((g * 2 + p) * 512, (g * 2 + p + 1) * 512)
                            dst = sb[:, cs].rearrange("y (h x) -> y h x", h=2)
                            nc.sync.dma_start(dst, src)
                    s = lpool.tile([128, 512], FP32, tag="s")
                    nc.vector.tensor_tensor(s[:], sb[:, 0:512], sb[:, 512:1024],
                                            AluOp.add)
                    ind = lpool.tile([128, 512], FP32, tag="ind")
                    nc.vector.tensor_scalar(ind[:], s[:], 0.0, None, AluOp.is_gt)
                    nc.vector.tensor_reduce(t_acc[:, RW + r:RW + r + 1], ind[:],
                                            axis=AX.X, op=AluOp.add)
                    den = lpool.tile([128, 1024], FP32, tag="den")
                    nc.vector.tensor_scalar(den[:], sb[:, 0:1024], EPS, None,
                                            AluOp.add)
                    rec = lpool.tile([128, 1024], FP32, tag="rec")
                    nc.vector.reciprocal(rec[:], den[:])
                    iwat = lpool.tile([128, 1024], FP32, tag="iwat")
                    nc.vector.tensor_tensor(iwat[:], rec[:], sb[:, 1024:2048],
                                            AluOp.mult)
                    scr2 = lpool.tile([128, 1024], FP32, tag="scr2")
                    nc.scalar.activation(scr2[:], iwat[:], AF.Square,
                                         accum_out=t_acc[:, r:r + 1])

                psf = lpsum.tile([1, 2 * RW], FP32)
                nc.tensor.matmul(psf[:], t_ones[:, 0:1], t_acc[:],
                                 start=True, stop=True)
                fin = lpool.tile([1, 2 * RW], FP32, tag="fin")
                nc.vector.tensor_copy(fin[:], psf[:])
                den2 = lpool.tile([1, R], FP32, tag="den2")
                nc.vector.tensor_scalar(den2[:], fin[:, RW:RW + R], EPS, None,
                                        AluOp.add)
                rec2 = lpool.tile([1, R], FP32, tag="rec2")
                nc.vector.reciprocal(rec2[:], den2[:])
                lossv = lpool.tile([1, R], FP32, tag="lossv")
                nc.vector.tensor_tensor(lossv[:], fin[:, 0:R], rec2[:],
                                        AluOp.mult)
                nc.sync.dma_start(out_loss[:], lossv[:])
    return nc


def prep_core_inputs(ev, flow_b, zg):
    """ev: [SHARD_len, 5] events of this core; flow_b: [BASE,H,W,2]."""
    nt = BASE * zg
    cap = zg * 128
    x, y, t, ts, p = (np.asarray(ev[:, i], dtype=np.float64) for i in range(5))
    zi = np.clip(np.floor(t).astype(np.int64), 0, BASE - 1)

    fx, fy, ft, fts, fp = (np.full(BASE * cap, v, dtype=np.float32)
                           for v in (-1000.0, -1000.0, 0.0, 0.0, 0.0))
    for z in range(BASE):
        m = np.where(zi == z)[0]
        assert len(m) <= cap, f"z-group {z} overflow: {len(m)} > {cap}"
        o = z * cap
        fx[o:o + len(m)] = x[m]
        fy[o:o + len(m)] = y[m]
        ft[o:o + len(m)] = t[m]
        fts[o:o + len(m)] = ts[m]
        fp[o:o + len(m)] = p[m]

    def lay(a):  # [BASE*cap] -> [128, nt]
        return np.ascontiguousarray(a.reshape(nt, 128).T)

    pq = np.clip(fp, 0.0, 1.0)
    d = {
        "ev_x": lay(fx),
        "ev_y": lay(fy),
        "ev_ts": lay(fts),
        "ev_negt": lay(-ft),
        "ev_xoff": lay(fx + 257.0 * pq),
        "ev_pm1": lay(257.0 * pq - 1.0),
        "ev_pp256": lay(257.0 * pq + 256.0),
    }
    # flow layout: [ylane, (z,yh), (c,x)]
    f = np.asarray(flow_b, dtype=np.float32)          # [z, y, x, c]
    f = f.reshape(BASE, 2, 128, W, 2)                 # [z, yh, ylane, x, c]
    f = np.transpose(f, (2, 0, 1, 4, 3))              # [ylane, z, yh, c, x]
    d["flow_in"] = np.ascontiguousarray(
        f.reshape(128, BASE * 2 * 2 * W)).reshape(128, 2 * BASE, 512)

    d["c_iota514"] = np.broadcast_to(
        np.arange(XW, dtype=np.float32), (128, XW)).copy()
    d["c_iota258"] = np.broadcast_to(
        np.arange(YW, dtype=np.float32), (128, YW)).copy()
    d["c_r12"] = np.broadcast_to(
        np.arange(RW, dtype=np.float32), (128, RW)).copy()
    d["c_ident"] = np.eye(128, dtype=np.float32)
    d["c_ones"] = np.ones((128, 2), dtype=np.float32)
    return d


_CACHE = {}


def _get_nc(zg, do_allreduce, ngroup, out_grids, use_for_i, num_devices,
            phases="ABL"):
    key = (zg, do_allreduce, ngroup, out_grids, use_for_i, num_devices, phases)
    if key not in _CACHE:
        nc = bacc.Bacc("TRN2", target_bir_lowering=False, debug=False,
                       num_devices=num_devices)
        build_kernel(nc, zg, do_allreduce, ngroup, out_grids, use_for_i, phases)
        nc.compile()
        _CACHE[key] = nc
    return _CACHE[key]


def kernel(events, flow_maps, base, zg=56, timeit=0):
    import time
    events = np.asarray(events)
    flow_maps = np.asarray(flow_maps)
    nc = _get_nc(zg, True, CORES_PER_B, False, True, NCORES)
    in_maps = []
    for c in range(NCORES):
        b, q = c // CORES_PER_B, c % CORES_PER_B
        ev = events[b, q * SHARD:(q + 1) * SHARD]
        in_maps.append(prep_core_inputs(ev, flow_maps[b], zg))
    res = run_bass_kernel_spmd(nc, in_maps, list(range(NCORES)))
    out = np.stack([res.results[0]["loss"][0], res.results[CORES_PER_B]["loss"][0]])
    out = out.astype(np.float32)
    if timeit:
        best = float("inf")
        for _ in range(timeit):
            t0 = time.time()
            run_bass_kernel_spmd(nc, in_maps, list(range(NCORES)))
            best = min(best, time.time() - t0)
        return out, best
    return out
